# revision 20
# baseline (speedup 1.0000x reference)
"""Trainium2 kernel for nn_Head_87660282511715 (retrieval_knn).

Pipeline (reference semantics):
  d2[p, j] = ||feat_p - bank_j||^2 over 8192 query pixels x 16384 bank rows
  per-pixel nearest distance -> 32x32 mask per batch -> bilinear 256x256 + gaussian blur
  per-batch: patch with max nearest-distance -> 9-NN softmax weight -> score

Device work (the 137 GFLOP part): for every pixel, min_j (|v_j|^2 - 2 f.v_j),
computed as a bf16 matmul (fp32 accumulate in PSUM) fused with a single-pass
DVE tensor_tensor_reduce (bias add + running min) per PSUM tile.

Sharding: the bank (16384 rows) is split over the 8 cores (2048 rows each);
every core sees all 8192 pixels. Host merges with an elementwise min (exact),
then runs the tiny epilogue in numpy, recomputing the few candidate argmax
patches exactly in fp64 so bf16 noise cannot flip the patch selection.
"""

import numpy as np
import ml_dtypes

import concourse.bass as bass
import concourse.bacc as bacc
import concourse.mybir as mybir
import concourse.tile as tile
from concourse.bass_utils import run_bass_kernel_spmd

N_CORES = 8
C = 512            # feature dim
NPIX = 8192        # 8 * 32 * 32 query pixels
NBANK = 16384
SHARD = NBANK // N_CORES      # bank rows per core
PT = 128                      # pixels per tile (PSUM partitions)
NPT = NPIX // PT              # 64 pixel tiles
KC = C // 128                 # 4 contraction chunks
NCHUNK = 512                  # candidate chunk (one PSUM bank)
CC = SHARD // NCHUNK          # 4 candidate chunks per core

B, H, W = 8, 32, 32
IMAGE = 256
SIGMA, RADIUS, K = 4.0, 16, 9
DELTA = 2.0   # d2-units refinement margin (measured bf16 noise max ~0.61)

_BIG = 3.0e38


def _build_bass():
    nc = bacc.Bacc("TRN2", target_bir_lowering=False)
    # fT packed host-side as [128, KC, NPIX] (partition-major); bank blob is
    # [128, KC+1, SHARD] with the |v|^2 hi/lo rows embedded in chunk KC.
    FOFF = KC * NPIX            # offset of bank chunks inside the blob
    WOFF = FOFF + KC * SHARD    # offset of the |v|^2 hi/lo rows
    BLOB = WOFF + SHARD
    blob = nc.dram_tensor("blob", [PT, BLOB], mybir.dt.bfloat16, kind="ExternalInput")
    m_out = nc.dram_tensor("m", [PT, NPT], mybir.dt.float32, kind="ExternalOutput")

    with tile.TileContext(nc) as tc:
        with tc.tile_pool(name="big", bufs=1) as big, \
             tc.tile_pool(name="psum", bufs=8, space="PSUM") as psum_pool, \
             tc.tile_pool(name="scratch", bufs=4) as scratch_pool:
            blob_sb = big.tile([PT, BLOB], mybir.dt.bfloat16)
            ones_sb = big.tile([2, PT], mybir.dt.bfloat16)
            m_sb = big.tile([PT, NPT], mybir.dt.float32)

            nc.sync.dma_start(blob_sb[:], blob[:])
            nc.vector.memset(ones_sb[:], 1.0)

            for pt in range(NPT):
                psums = [psum_pool.tile([PT, NCHUNK], mybir.dt.float32, name="ps", tag="ps")
                         for _ in range(CC)]
                for k in range(KC):
                    lhsT = blob_sb[:, k * NPIX + pt * PT:k * NPIX + (pt + 1) * PT]
                    for j in range(CC):
                        o = FOFF + k * SHARD + j * NCHUNK
                        nc.tensor.matmul(
                            psums[j][:], lhsT, blob_sb[:, o:o + NCHUNK],
                            start=(k == 0), stop=False)
                # bias pass: accumulate |v_j|^2 (split hi+lo in bf16) via ones^T @ w2
                for j in range(CC):
                    o = WOFF + j * NCHUNK
                    nc.tensor.matmul(
                        psums[j][:], ones_sb[:], blob_sb[0:2, o:o + NCHUNK],
                        start=False, stop=True)
                mpart = scratch_pool.tile([PT, CC], mybir.dt.float32, name="mp", tag="mp")
                for j in range(CC):
                    nc.vector.tensor_reduce(
                        mpart[:, j:j + 1], psums[j][:],
                        axis=mybir.AxisListType.X, op=mybir.AluOpType.min)
                nc.vector.tensor_reduce(
                    m_sb[:, pt:pt + 1], mpart[:],
                    axis=mybir.AxisListType.X, op=mybir.AluOpType.min)

            # single-writer staging copy (same engine as the reduces -> program
            # order, no per-writer DMA sync waits)
            m2_sb = big.tile([PT, NPT], mybir.dt.float32)
            nc.vector.tensor_copy(m2_sb[:], m_sb[:])
            nc.sync.dma_start(m_out[:], m2_sb[:])
    nc.finalize()
    return nc


_NC_CACHE = None


def _device_min(f16, bank16, w32):
    """Returns min_j (|v_j|^2 - 2 f.v_j) per pixel, via 8 TRN2 cores."""
    global _NC_CACHE
    if _NC_CACHE is None:
        _NC_CACHE = _build_bass()
    nc = _NC_CACHE

    # blob[p, k*NPIX + x] = f16.T[k*128+p, x]  (partition-major interleave),
    # then bank chunks, then the |v|^2 hi/lo rows.
    fT4 = np.ascontiguousarray(
        f16.T.reshape(KC, PT, NPIX).transpose(1, 0, 2)).reshape(PT, KC * NPIX)
    bank_m2 = (bank16.astype(np.float32) * -2.0).astype(ml_dtypes.bfloat16)  # exact
    w_hi = w32.astype(ml_dtypes.bfloat16)
    w_lo = (w32 - w_hi.astype(np.float32)).astype(ml_dtypes.bfloat16)
    in_maps = []
    for s in range(N_CORES):
        sl = slice(s * SHARD, (s + 1) * SHARD)
        bpart = np.zeros((PT, KC + 1, SHARD), dtype=ml_dtypes.bfloat16)
        bpart[:, :KC, :] = bank_m2[sl].T.reshape(KC, PT, SHARD).transpose(1, 0, 2)
        bpart[0, KC, :] = w_hi[sl]
        bpart[1, KC, :] = w_lo[sl]
        blob = np.concatenate([fT4, bpart.reshape(PT, -1)], axis=1)
        in_maps.append({"blob": np.ascontiguousarray(blob)})

    res = run_bass_kernel_spmd(nc, in_maps, core_ids=list(range(N_CORES)))
    # m[lane, pt] -> pixel pt*128 + lane; global min across bank shards
    m = np.stack([r["m"].T.reshape(NPIX) for r in res.results])
    return m.min(axis=0)


def _axis_operator():
    """[256, 32] operator: bilinear 32->256 (half-pixel) then reflect-101 gaussian."""
    A = np.zeros((IMAGE, W), dtype=np.float64)
    s = W / IMAGE
    for i in range(IMAGE):
        c = (i + 0.5) * s - 0.5
        i0 = int(np.floor(c))
        t = c - i0
        A[i, min(max(i0, 0), W - 1)] += 1 - t
        A[i, min(max(i0 + 1, 0), W - 1)] += t
    i = np.arange(2 * RADIUS + 1, dtype=np.float64) - RADIUS
    g = np.exp(-(i ** 2) / (2.0 * SIGMA ** 2))
    g = g / g.sum()
    P = np.zeros((IMAGE + 2 * RADIUS, IMAGE), dtype=np.float64)
    for j in range(IMAGE + 2 * RADIUS):
        idx = j - RADIUS
        if idx < 0:
            idx = -idx
        if idx > IMAGE - 1:
            idx = 2 * (IMAGE - 1) - idx
        P[j, idx] = 1.0
    Bm = np.zeros((IMAGE, IMAGE + 2 * RADIUS), dtype=np.float64)
    for ii in range(IMAGE):
        Bm[ii, ii:ii + 2 * RADIUS + 1] = g
    return Bm @ P @ A


_T_OP = None


def kernel(inputs: np.ndarray, feature_vector: np.ndarray):
    global _T_OP
    f32 = np.ascontiguousarray(inputs.reshape(NPIX, C).astype(np.float32))
    v32 = np.ascontiguousarray(feature_vector.astype(np.float32))

    f16 = f32.astype(ml_dtypes.bfloat16)
    v16 = v32.astype(ml_dtypes.bfloat16)
    w32 = (v16.astype(np.float32) ** 2).sum(axis=1)       # |v_j|^2 of bf16 bank

    m = _device_min(f16, v16, w32)                        # [8192]
    d2min = m + (f32 ** 2).sum(axis=1)                    # bf16-accurate d2
    dist = np.sqrt(np.maximum(d2min, 0.0), dtype=np.float32)

    # ---- per-batch argmax patch with exact fp64 refinement ----
    f64 = f32.astype(np.float64)
    v64 = v32.astype(np.float64)
    w64 = (v64 ** 2).sum(axis=1)
    d2b = d2min.reshape(B, H * W)
    score_out = np.empty((B, 1), dtype=np.float32)
    for b in range(B):
        cand = np.nonzero(d2b[b] > d2b[b].max() - DELTA)[0]
        best_val, best_row = -np.inf, None
        for p in cand:
            pix = b * H * W + p
            row = w64 + (f64[pix] ** 2).sum() - 2.0 * (v64 @ f64[pix])
            mn = row.min()
            if mn > best_val:
                best_val, best_row = mn, row
        conf = np.sqrt(np.maximum(np.sort(best_row)[:K], 0.0))
        e = np.exp(conf)
        weight = 1.0 - e.max() / e.sum()
        score_out[b, 0] = np.float32(np.sqrt(max(best_val, 0.0)) * weight)

    # ---- mask: resize + blur as one exact linear operator per axis ----
    if _T_OP is None:
        _T_OP = _axis_operator()
    T = _T_OP
    mask32 = dist.reshape(B, H, W).astype(np.float64)
    mask = np.einsum('ij,bjk,lk->bil', T, mask32, T)
    mask = mask.astype(np.float32).reshape(B, IMAGE, IMAGE, 1)
    return score_out, mask


# revision 21
# speedup vs baseline: 1.0614x; 1.0614x over previous
"""Trainium2 kernel for nn_Head_87660282511715 (retrieval_knn).

Pipeline (reference semantics):
  d2[p, j] = ||feat_p - bank_j||^2 over 8192 query pixels x 16384 bank rows
  per-pixel nearest distance -> 32x32 mask per batch -> bilinear 256x256 + gaussian blur
  per-batch: patch with max nearest-distance -> 9-NN softmax weight -> score

Device work (the 137 GFLOP part): for every pixel, min_j (|v_j|^2 - 2 f.v_j),
computed as a bf16 matmul (fp32 accumulate in PSUM) fused with a single-pass
DVE tensor_tensor_reduce (bias add + running min) per PSUM tile.

Sharding: the bank (16384 rows) is split over the 8 cores (2048 rows each);
every core sees all 8192 pixels. Host merges with an elementwise min (exact),
then runs the tiny epilogue in numpy, recomputing the few candidate argmax
patches exactly in fp64 so bf16 noise cannot flip the patch selection.
"""

import numpy as np
import ml_dtypes

import concourse.bass as bass
import concourse.bacc as bacc
import concourse.mybir as mybir
import concourse.tile as tile
from concourse.bass_utils import run_bass_kernel_spmd

N_CORES = 8
C = 512            # feature dim
NPIX = 8192        # 8 * 32 * 32 query pixels
NBANK = 16384
SHARD = NBANK // N_CORES      # bank rows per core
PT = 128                      # pixels per tile (PSUM partitions)
NPT = NPIX // PT              # 64 pixel tiles
KC = C // 128                 # 4 contraction chunks
NCHUNK = 512                  # candidate chunk (one PSUM bank)
CC = SHARD // NCHUNK          # 4 candidate chunks per core

B, H, W = 8, 32, 32
IMAGE = 256
SIGMA, RADIUS, K = 4.0, 16, 9
DELTA = 2.0   # d2-units refinement margin (measured bf16 noise max ~0.61)

_BIG = 3.0e38


def _build_bass():
    nc = bacc.Bacc("TRN2", target_bir_lowering=False)
    # fT packed host-side as [128, KC, NPIX] (partition-major); bank blob is
    # [128, KC+1, SHARD] with the |v|^2 hi/lo rows embedded in chunk KC.
    FOFF = KC * NPIX            # offset of bank chunks inside the blob
    WOFF = FOFF + KC * SHARD    # offset of the |v|^2 hi/lo rows
    BLOB = WOFF + SHARD
    blob = nc.dram_tensor("blob", [PT, BLOB], mybir.dt.bfloat16, kind="ExternalInput")
    m_out = nc.dram_tensor("m", [PT, NPT], mybir.dt.float32, kind="ExternalOutput")

    with tile.TileContext(nc) as tc:
        with tc.tile_pool(name="big", bufs=1) as big, \
             tc.tile_pool(name="psum", bufs=8, space="PSUM") as psum_pool, \
             tc.tile_pool(name="scratch", bufs=4) as scratch_pool:
            blob_sb = big.tile([PT, BLOB], mybir.dt.bfloat16)
            ones_sb = big.tile([2, PT], mybir.dt.bfloat16)
            m_sb = big.tile([PT, NPT], mybir.dt.float32)

            # bank+bias first (every matmul needs it), then feat in pixtile-
            # ordered pieces so the first matmul group starts ~25us earlier;
            # HWDGE DMAs on the SP ring drain in program order.
            nc.sync.dma_start(blob_sb[:, FOFF:BLOB], blob[:, FOFF:BLOB])
            bounds = [0, 8, 24, NPT]
            for a, b in zip(bounds, bounds[1:]):
                for k in range(KC):
                    o0, o1 = k * NPIX + a * PT, k * NPIX + b * PT
                    nc.sync.dma_start(blob_sb[:, o0:o1], blob[:, o0:o1])
            nc.vector.memset(ones_sb[:], 1.0)

            for pt in range(NPT):
                psums = [psum_pool.tile([PT, NCHUNK], mybir.dt.float32, name="ps", tag="ps")
                         for _ in range(CC)]
                for k in range(KC):
                    lhsT = blob_sb[:, k * NPIX + pt * PT:k * NPIX + (pt + 1) * PT]
                    for j in range(CC):
                        o = FOFF + k * SHARD + j * NCHUNK
                        nc.tensor.matmul(
                            psums[j][:], lhsT, blob_sb[:, o:o + NCHUNK],
                            start=(k == 0), stop=False)
                # bias pass: accumulate |v_j|^2 (split hi+lo in bf16) via ones^T @ w2
                for j in range(CC):
                    o = WOFF + j * NCHUNK
                    nc.tensor.matmul(
                        psums[j][:], ones_sb[:], blob_sb[0:2, o:o + NCHUNK],
                        start=False, stop=True)
                mpart = scratch_pool.tile([PT, CC], mybir.dt.float32, name="mp", tag="mp")
                for j in range(CC):
                    nc.vector.tensor_reduce(
                        mpart[:, j:j + 1], psums[j][:],
                        axis=mybir.AxisListType.X, op=mybir.AluOpType.min)
                nc.vector.tensor_reduce(
                    m_sb[:, pt:pt + 1], mpart[:],
                    axis=mybir.AxisListType.X, op=mybir.AluOpType.min)

            # single-writer staging copy (same engine as the reduces -> program
            # order, no per-writer DMA sync waits)
            m2_sb = big.tile([PT, NPT], mybir.dt.float32)
            nc.vector.tensor_copy(m2_sb[:], m_sb[:])
            nc.sync.dma_start(m_out[:], m2_sb[:])
    nc.finalize()
    return nc


_NC_CACHE = None


def _device_min(f16, bank16, w32):
    """Returns min_j (|v_j|^2 - 2 f.v_j) per pixel, via 8 TRN2 cores."""
    global _NC_CACHE
    if _NC_CACHE is None:
        _NC_CACHE = _build_bass()
    nc = _NC_CACHE

    # blob[p, k*NPIX + x] = f16.T[k*128+p, x]  (partition-major interleave),
    # then bank chunks, then the |v|^2 hi/lo rows.
    fT4 = np.ascontiguousarray(
        f16.T.reshape(KC, PT, NPIX).transpose(1, 0, 2)).reshape(PT, KC * NPIX)
    bank_m2 = (bank16.astype(np.float32) * -2.0).astype(ml_dtypes.bfloat16)  # exact
    w_hi = w32.astype(ml_dtypes.bfloat16)
    w_lo = (w32 - w_hi.astype(np.float32)).astype(ml_dtypes.bfloat16)
    in_maps = []
    for s in range(N_CORES):
        sl = slice(s * SHARD, (s + 1) * SHARD)
        bpart = np.zeros((PT, KC + 1, SHARD), dtype=ml_dtypes.bfloat16)
        bpart[:, :KC, :] = bank_m2[sl].T.reshape(KC, PT, SHARD).transpose(1, 0, 2)
        bpart[0, KC, :] = w_hi[sl]
        bpart[1, KC, :] = w_lo[sl]
        blob = np.concatenate([fT4, bpart.reshape(PT, -1)], axis=1)
        in_maps.append({"blob": np.ascontiguousarray(blob)})

    res = run_bass_kernel_spmd(nc, in_maps, core_ids=list(range(N_CORES)))
    # m[lane, pt] -> pixel pt*128 + lane; global min across bank shards
    m = np.stack([r["m"].T.reshape(NPIX) for r in res.results])
    return m.min(axis=0)


def _axis_operator():
    """[256, 32] operator: bilinear 32->256 (half-pixel) then reflect-101 gaussian."""
    A = np.zeros((IMAGE, W), dtype=np.float64)
    s = W / IMAGE
    for i in range(IMAGE):
        c = (i + 0.5) * s - 0.5
        i0 = int(np.floor(c))
        t = c - i0
        A[i, min(max(i0, 0), W - 1)] += 1 - t
        A[i, min(max(i0 + 1, 0), W - 1)] += t
    i = np.arange(2 * RADIUS + 1, dtype=np.float64) - RADIUS
    g = np.exp(-(i ** 2) / (2.0 * SIGMA ** 2))
    g = g / g.sum()
    P = np.zeros((IMAGE + 2 * RADIUS, IMAGE), dtype=np.float64)
    for j in range(IMAGE + 2 * RADIUS):
        idx = j - RADIUS
        if idx < 0:
            idx = -idx
        if idx > IMAGE - 1:
            idx = 2 * (IMAGE - 1) - idx
        P[j, idx] = 1.0
    Bm = np.zeros((IMAGE, IMAGE + 2 * RADIUS), dtype=np.float64)
    for ii in range(IMAGE):
        Bm[ii, ii:ii + 2 * RADIUS + 1] = g
    return Bm @ P @ A


_T_OP = None


def kernel(inputs: np.ndarray, feature_vector: np.ndarray):
    global _T_OP
    f32 = np.ascontiguousarray(inputs.reshape(NPIX, C).astype(np.float32))
    v32 = np.ascontiguousarray(feature_vector.astype(np.float32))

    f16 = f32.astype(ml_dtypes.bfloat16)
    v16 = v32.astype(ml_dtypes.bfloat16)
    w32 = (v16.astype(np.float32) ** 2).sum(axis=1)       # |v_j|^2 of bf16 bank

    m = _device_min(f16, v16, w32)                        # [8192]
    d2min = m + (f32 ** 2).sum(axis=1)                    # bf16-accurate d2
    dist = np.sqrt(np.maximum(d2min, 0.0), dtype=np.float32)

    # ---- per-batch argmax patch with exact fp64 refinement ----
    f64 = f32.astype(np.float64)
    v64 = v32.astype(np.float64)
    w64 = (v64 ** 2).sum(axis=1)
    d2b = d2min.reshape(B, H * W)
    score_out = np.empty((B, 1), dtype=np.float32)
    for b in range(B):
        cand = np.nonzero(d2b[b] > d2b[b].max() - DELTA)[0]
        best_val, best_row = -np.inf, None
        for p in cand:
            pix = b * H * W + p
            row = w64 + (f64[pix] ** 2).sum() - 2.0 * (v64 @ f64[pix])
            mn = row.min()
            if mn > best_val:
                best_val, best_row = mn, row
        conf = np.sqrt(np.maximum(np.sort(best_row)[:K], 0.0))
        e = np.exp(conf)
        weight = 1.0 - e.max() / e.sum()
        score_out[b, 0] = np.float32(np.sqrt(max(best_val, 0.0)) * weight)

    # ---- mask: resize + blur as one exact linear operator per axis ----
    if _T_OP is None:
        _T_OP = _axis_operator()
    T = _T_OP
    mask32 = dist.reshape(B, H, W).astype(np.float64)
    mask = np.einsum('ij,bjk,lk->bil', T, mask32, T)
    mask = mask.astype(np.float32).reshape(B, IMAGE, IMAGE, 1)
    return score_out, mask
